# revision 24
# baseline (speedup 1.0000x reference)
"""Trainium2 Bass kernel for BandSplitModule (masked LN per band + weight-normed Linear).

Strategy (v2 — LN folded into the matmul, K-major everything):
  - Data-parallel over T (2048 = 8 cores x 256). No collectives.
  - Host folds weight-norm + LN affine into W2 = (g*v/||v||)*(gamma*mask),
    bias2 = W@(beta*mask) + bias, then CENTERS the weights:
    W' = W2 - outer(rowsum(W2)/n, mask) so that W'@x = W2@(x - mean).
    Device computes z = rs * (W' @ x) + bias2 with rs = 1/sqrt(var+eps).
  - x is shipped K-major: [33 chunks, 128 k, B*TLOC] bf16. The main matmul
    uses x chunks as the PE stationary (out [t, E] per band), so the final
    scale by rs[t] is a per-partition scalar op.
  - Per-band stats are batched on the TensorEngine: mask/n-stationary
    matmuls stream x and x*x, accumulating Sx=[37,1024], Sxx=[37,1024]
    in PSUM across all 33 chunks. var = Sxx - Sx^2, rs = 1/sqrt(var+eps),
    then 8 tiny PE transposes give rs in [t,37] layout.
  - Output drains (psum->sbuf, *rs) round-robin across Scalar/Vector/GpSimd;
    bias2 is added with one whole-slab [128, 37*128] vector add per slab.
  - Runtime band_start/band_width are baked into the compiled program
    (compilation happens inside kernel(); results cached per band structure).
"""
import numpy as np

B, C, F, T, E = 4, 2, 1025, 2048, 128
MAX_BW = 65
NB = 37
EPS = 1e-5
NCORES = 8
TLOC = T // NCORES  # 256
D = C * MAX_BW * 2  # 260
KF = 4 * F  # 4100 global features (freq-major: k = 4f + 2c + r)
NC = 33  # k chunks of 128
K = NC * 128  # 4224 padded
NSLAB = B * (TLOC // 128)  # 8 output slabs per core
ZW = NB * E  # 4736 output slab width

LAST_EXEC_NS = None

_PLAN_CACHE = {}


def _ensure_trace_hook():
    """Install the antenv.axon_hooks NTFF-profile shim (missing on this image)
    so run_bass_kernel_spmd(trace=True) can capture HW exec time. Fully
    optional — any failure leaves the plain execution path untouched."""
    try:
        import sys, types

        if "antenv.axon_hooks" not in sys.modules:
            mod = types.ModuleType("antenv.axon_hooks")
            _h = {"hook": None}
            mod.set_axon_ntff_profile_hook = lambda h: _h.__setitem__("hook", h)
            mod.get_axon_ntff_profile_hook = lambda: _h["hook"]
            sys.modules["antenv.axon_hooks"] = mod
            try:
                import antenv

                antenv.axon_hooks = mod
            except Exception:
                pass
            try:
                from trn_agent_boot.trn_boot import _ntff_profile_via_ctypes

                hook = _ntff_profile_via_ctypes("/opt/axon/libaxon_pjrt.so")
                if hook is not None:
                    mod.set_axon_ntff_profile_hook(hook)
            except Exception:
                pass
        import concourse.bass_utils as bu

        if not getattr(bu, "_offline_upload_patch", False):
            bu.upload_artifacts = lambda tmpdir: tmpdir
            bu._offline_upload_patch = True
    except Exception:
        pass


def _feature_perm():
    # new index (k,c,r) -> reference index (c,k,r), within one band
    kk, cc, rr = np.meshgrid(
        np.arange(MAX_BW), np.arange(C), np.arange(2), indexing="ij"
    )
    new_i = (kk * 4 + cc * 2 + rr).reshape(-1)
    src_i = (cc * (MAX_BW * 2) + kk * 2 + rr).reshape(-1)
    perm = np.empty(D, np.int64)
    perm[new_i] = src_i
    return perm


def _band_rows(starts, widths):
    """Per band: the global k rows its (clipped) features map to.
    Returns list of arrays rows[n] of length 4*w_n (duplicates where the
    reference's freq clip at F-1 folds several kk onto the same row)."""
    rows = []
    for s, w in zip(starts, widths):
        kk = np.arange(int(w))
        f = np.clip(int(s) + kk, 0, F - 1)
        r4 = (4 * f[:, None] + np.arange(4)[None, :]).reshape(-1)  # (k-major)
        rows.append(r4)
    return rows


def _fold_weights(ln_gamma, ln_beta, v, g, bias, starts, widths):
    """Returns Wg [K, NB*E] f32 (centered, k-major global rows), bias2 [NB,E],
    maskn [K, NB] f32 (1/n per valid row)."""
    karr = np.arange(MAX_BW)
    bw_mask = karr[None, :] < widths[:, None]
    fm = (
        np.broadcast_to(bw_mask[:, None, :, None], (NB, C, MAX_BW, 2))
        .reshape(NB, D)
        .astype(np.float32)
    )
    vnorm = np.sqrt((v * v).sum(-1, keepdims=True))
    vnorm = np.where(vnorm == 0, 1.0, vnorm)
    W = g[..., None] * v / vnorm
    W2 = W * (ln_gamma * fm)[:, None, :]
    bias2 = np.einsum("ned,nd->ne", W, ln_beta * fm) + bias
    W2p = W2[:, :, _feature_perm()]  # [NB, E, D] in (k,c,r) order

    rows = _band_rows(starts, widths)
    Wg = np.zeros((K, NB * E), np.float32)
    maskn = np.zeros((K, NB), np.float32)
    for n in range(NB):
        w = int(widths[n])
        if w == 0:
            continue
        nfeat = float(4 * w)
        wsum2 = W2p[n, :, : 4 * w].sum(axis=1)  # [E]
        Wc = W2p[n, :, : 4 * w].T - wsum2[None, :] / nfeat  # [4w, E] centered
        np.add.at(Wg, (rows[n], slice(n * E, (n + 1) * E)), Wc)
        np.add.at(maskn, (rows[n], n), 1.0 / nfeat)
    return Wg, bias2, maskn


def _plan_chunks(starts, widths):
    """Per 128-row chunk: matmul groups [(wcol, ncols, bands, start, stop)].
    Bands fully inside a chunk are merged pairwise (psum col range is 256)."""
    ranges = []
    for s, w in zip(starts, widths):
        lo = 4 * min(int(s), F - 1)
        hi = 4 * min(int(s) + int(w), F)
        ranges.append((lo, hi))
    chunk_groups = []
    wcol = 0
    for c in range(NC):
        clo, chi = 128 * c, 128 * c + 128
        groups = []
        run = []  # accumulating mergeable full bands
        for n in range(NB):
            lo, hi = ranges[n]
            if hi <= lo or hi <= clo or lo >= chi:
                continue
            full = lo >= clo and hi <= chi
            # merge only full bands that share the same 4-band psum tile
            if full and len(run) < 2 and (not run or run[-1] // 4 == n // 4):
                run.append(n)
                continue
            if run:
                groups.append((wcol, 128 * len(run), tuple(run), True, True))
                wcol += 128 * len(run)
                run = []
            if full:
                run.append(n)
            else:
                groups.append((wcol, 128, (n,), lo >= clo, hi <= chi))
                wcol += 128
        if run:
            groups.append((wcol, 128 * len(run), tuple(run), True, True))
            wcol += 128 * len(run)
        chunk_groups.append(groups)
    return chunk_groups, wcol


def _plan_bgroups(starts, widths, chunk_groups):
    """Split bands into <=32-band groups with contiguous chunk ranges so
    stats/rs/z pipelines per group. Returns list of dicts."""
    ranges = []
    for s, w in zip(starts, widths):
        lo = 4 * min(int(s), F - 1)
        hi = 4 * min(int(s) + int(w), F)
        ranges.append((max(0, lo // 128), max(0, (hi + 127) // 128 - 1)))
    # three band groups (sizes <= 32 so they fit PE quadrant col offsets)
    cuts = [0, 20, 32, NB]
    bgroups = [list(range(cuts[i], cuts[i + 1])) for i in range(3) if cuts[i] < cuts[i + 1]]
    out = []
    for gi, bands in enumerate(bgroups):
        c0 = min(ranges[n][0] for n in bands)
        c1 = max(ranges[n][1] for n in bands)
        out.append(
            dict(
                gi=gi,
                bands=bands,
                nb=len(bands),
                b0=bands[0],
                poff=32 * gi,  # partition offset in shared stats tiles
                c0=c0,
                c1=c1,
            )
        )
    assert len(out) <= 4
    return out


def _pack_wt(Wg, chunk_groups, wcols):
    Wt = np.zeros((128, wcols), np.float32)
    for c, groups in enumerate(chunk_groups):
        sl = Wg[128 * c : 128 * c + 128]
        for wcol, ncols, bands, _, _ in groups:
            off = wcol
            for n in bands:
                Wt[:, off : off + E] = sl[:, n * E : (n + 1) * E]
                off += E
    return Wt


def _prep_x(x):
    """x [B,C,F,T,2] f32 -> [NCORES, NC, 128, B*TLOC] bf16 (k-major chunks)."""
    import ml_dtypes

    xr = np.ascontiguousarray(x.transpose(2, 1, 4, 0, 3)).reshape(KF, B, T)
    xk = np.zeros((K, B, T), ml_dtypes.bfloat16)
    xk[:KF] = xr
    # cols per core: b-major (b*TLOC + t)
    xk = xk.reshape(K, B, NCORES, TLOC).transpose(2, 0, 1, 3)
    xk = np.ascontiguousarray(xk.reshape(NCORES, NC, 128, B * TLOC))
    return xk


class _Balance:
    """Greedy codegen-time engine load balancer (cost in ns, approximate)."""

    def __init__(self):
        self.load = {"v": 0.0, "s": 0.0, "g": 0.0}

    def pick(self, costs):
        eng = min(costs, key=lambda e: self.load[e] + costs[e])
        self.load[eng] += costs[eng]
        return eng


def _build_program(chunk_groups, wcols, bgroups, widths):
    import concourse.bass as bass
    import concourse.bacc as bacc
    import concourse.tile as tile
    from concourse import mybir
    from concourse.masks import make_identity
    from contextlib import ExitStack

    f32 = mybir.dt.float32
    bf16 = mybir.dt.bfloat16
    TC = 1024  # B * TLOC columns per core
    nc = bacc.Bacc()
    x_ext = nc.declare_dram_parameter("xk", [NC, 128, TC], bf16, isOutput=False)
    wt_ext = nc.declare_dram_parameter("wt", [128, wcols], bf16, isOutput=False)
    mk_ext = nc.declare_dram_parameter("mk", [128, NC * NB], bf16, isOutput=False)
    b2_ext = nc.declare_dram_parameter("b2", [128, ZW], bf16, isOutput=False)
    z_ext = nc.declare_dram_parameter("out", [NSLAB, 128, ZW], bf16, isOutput=True)

    bal = _Balance()

    with ExitStack() as ctx:
        tc = ctx.enter_context(tile.TileContext(nc))
        consts = ctx.enter_context(tc.tile_pool(name="consts", bufs=1))
        xpool = ctx.enter_context(tc.tile_pool(name="x", bufs=1))
        x2pool = ctx.enter_context(tc.tile_pool(name="x2", bufs=6))
        rspool = ctx.enter_context(tc.tile_pool(name="rs", bufs=1))
        zspool = ctx.enter_context(tc.tile_pool(name="zs", bufs=3))
        st_psum = ctx.enter_context(tc.tile_pool(name="st", bufs=1, space="PSUM"))
        z_psum = ctx.enter_context(tc.tile_pool(name="zq", bufs=3, space="PSUM"))
        rs_psum = ctx.enter_context(tc.tile_pool(name="rp", bufs=1, space="PSUM"))

        ident = consts.tile([128, 128], f32)
        make_identity(nc, ident)
        eps_t = consts.tile([128, 1], f32)
        nc.vector.memset(eps_t, EPS)

        # DMA order: mask first (stats lhsT), then the first group's x chunks,
        # then weights/bias, then the remaining x chunks.
        mk_sb = consts.tile([128, NC * NB], bf16)
        nc.sync.dma_start(out=mk_sb, in_=mk_ext[:, :])
        xt = [None] * NC
        early = list(range(bgroups[0]["c0"], bgroups[0]["c1"] + 1))
        order = early + [c for c in range(NC) if c not in early]
        for i, c in enumerate(order):
            t = xpool.tile([128, TC], bf16, tag=f"x{c}", name=f"x{c}")
            nc.sync.dma_start(out=t, in_=x_ext[c])
            xt[c] = t
            if i == 3:
                wt_sb = consts.tile([128, wcols], bf16)
                nc.sync.dma_start(out=wt_sb, in_=wt_ext[:, :])
                b2_sb = consts.tile([128, ZW], bf16)
                nc.sync.dma_start(out=b2_sb, in_=b2_ext[:, :])

        # Shared stats psum tiles; group gi uses partitions [32*gi, 32*gi+nb)
        Sx0 = st_psum.tile([128, 512], f32, tag="Sx0")
        Sx1 = st_psum.tile([128, 512], f32, tag="Sx1")
        Sxx0 = st_psum.tile([128, 512], f32, tag="Sxx0")
        Sxx1 = st_psum.tile([128, 512], f32, tag="Sxx1")
        var_t = rspool.tile([128, TC], f32, tag="var")
        sqr_t = rspool.tile([128, TC], f32, tag="sqr")

        def emit_stats(g):
            poff, nb, b0 = g["poff"], g["nb"], g["b0"]
            for c in range(g["c0"], g["c1"] + 1):
                x2 = x2pool.tile([128, TC], bf16, tag="x2", name="x2")
                if c >= NC - 8:
                    eng = "g"  # late chunks: gpsimd has slack by then
                    bal.load["g"] += 2100
                else:
                    eng = bal.pick({"v": 1100, "s": 1100})
                if eng == "v":
                    nc.vector.tensor_mul(x2, xt[c], xt[c])
                elif eng == "g":
                    nc.gpsimd.tensor_mul(x2, xt[c], xt[c])
                else:
                    nc.scalar.square(out=x2, in_=xt[c])
                mk_c = mk_sb[:, c * NB + b0 : c * NB + b0 + nb]
                st = c == g["c0"]
                sp = c == g["c1"]
                o = slice(poff, poff + nb)
                nc.tensor.matmul(Sx0[o, :], lhsT=mk_c, rhs=xt[c][:, 0:512], start=st, stop=sp)
                nc.tensor.matmul(Sx1[o, :], lhsT=mk_c, rhs=xt[c][:, 512:1024], start=st, stop=sp)
                nc.tensor.matmul(Sxx0[o, :], lhsT=mk_c, rhs=x2[:, 0:512], start=st, stop=sp)
                nc.tensor.matmul(Sxx1[o, :], lhsT=mk_c, rhs=x2[:, 512:1024], start=st, stop=sp)

        def emit_rs(g):
            """var = Sxx - Sx^2; invrs = sqrt(var+eps); rsT[s] = 1/invrs^T."""
            poff, nb = g["poff"], g["nb"]
            o = slice(poff, poff + nb)
            rsT = []
            for h, (Sx, Sxx) in enumerate(((Sx0, Sxx0), (Sx1, Sxx1))):
                sl = slice(512 * h, 512 * h + 512)
                nc.scalar.square(out=var_t[o, sl], in_=Sx[o, :])
                nc.vector.tensor_sub(var_t[o, sl], Sxx[o, :], var_t[o, sl])
                nc.scalar.activation(
                    out=sqr_t[o, sl],
                    in_=var_t[o, sl],
                    func=mybir.ActivationFunctionType.Sqrt,
                    bias=eps_t[o, :],
                    scale=1.0,
                )
                for s in range(4 * h, 4 * h + 4):
                    rp = rs_psum.tile([128, 32], f32, tag="rp", name="rp")
                    nc.tensor.transpose(
                        out=rp[:, :nb],
                        in_=sqr_t[o, 128 * s : 128 * s + 128],
                        identity=ident[o, poff : poff + nb],
                    )
                    rt = rspool.tile([128, 32], f32, tag=f"rsT{g['gi']}_{s}", name="rt")
                    nc.vector.reciprocal(out=rt[:, :nb], in_=rp[:, :nb])
                    rsT.append(rt)
            return rsT

        def emit_z(g, rsT):
            b0, nb = g["b0"], g["nb"]
            gw = nb * E
            gcol0 = b0 * E
            bandset = set(g["bands"])
            # quads of 4 bands sharing one [128,512] psum tile
            quad_bands = {}
            for n in g["bands"]:
                if widths[n] > 0:
                    quad_bands.setdefault(n // 4, []).append(n)
            # per chunk, entries of this group
            centries = []
            for c in range(g["c0"], g["c1"] + 1):
                ent = [e for e in chunk_groups[c] if e[2][0] in bandset]
                centries.append((c, ent))
            for s in range(NSLAB):
                toff = 128 * s
                zg = zspool.tile([128, gw], bf16, tag=f"zg{g['gi']}", name="zg")
                qtile = {}
                qstops = {q: 0 for q in quad_bands}
                nstops = {q: len(v) for q, v in quad_bands.items()}
                for c, ent in centries:
                    x_c = xt[c][:, toff : toff + 128]
                    for wcol, ncols, bands, st, sp in ent:
                        q = bands[0] // 4
                        if q not in qtile:
                            qtile[q] = z_psum.tile([128, 512], f32, tag="zq", name="zq")
                        zp = qtile[q]
                        pc = (bands[0] % 4) * 128
                        nc.tensor.matmul(
                            zp[:, pc : pc + ncols],
                            lhsT=x_c,
                            rhs=wt_sb[:, wcol : wcol + ncols],
                            start=st,
                            stop=sp,
                        )
                        if sp:
                            qstops[q] += len(bands)
                            if qstops[q] == nstops[q]:
                                # scaled per-band drains psum->sbuf
                                for n in quad_bands[q]:
                                    dst = zg[:, (n - b0) * E : (n - b0 + 1) * E]
                                    src = zp[:, (n % 4) * E : (n % 4) * E + E]
                                    sc = rsT[s][:, n - b0 : n - b0 + 1]
                                    eng = bal.pick({"v": 394, "s": 512})
                                    if eng == "v":
                                        nc.vector.tensor_scalar(
                                            dst, src, sc, None, mybir.AluOpType.mult
                                        )
                                    else:
                                        nc.scalar.mul(out=dst, in_=src, mul=sc)
                # bias + store (gpsimd only for early big slabs, off the tail)
                if g["gi"] == 0 and s < 5:
                    bal.load["g"] += 4300
                    nc.gpsimd.tensor_add(zg, zg, b2_sb[:, gcol0 : gcol0 + gw])
                else:
                    bal.load["v"] += 60 + gw * 0.55
                    nc.vector.tensor_add(zg, zg, b2_sb[:, gcol0 : gcol0 + gw])
                nc.sync.dma_start(out=z_ext[s][:, gcol0 : gcol0 + gw], in_=zg)

        # ---- pipelined emission: stats(g0), rs(g0), stats(g1), z(g0), ... ----
        # rs(g) is emitted right after stats(g) so its small chain gets queue
        # priority over the next group's squares/stats on Scalar/Vector.
        emit_stats(bgroups[0])
        for gi, g in enumerate(bgroups):
            rsT = emit_rs(g)
            if gi + 1 < len(bgroups):
                emit_stats(bgroups[gi + 1])
            emit_z(g, rsT)
    nc.compile()
    return nc


def kernel(x, ln_gamma, ln_beta, v, g, bias, band_start, band_width):
    global LAST_EXEC_NS
    _ensure_trace_hook()
    from concourse.bass_utils import run_bass_kernel_spmd
    import ml_dtypes

    x = np.asarray(x, np.float32)
    ln_gamma = np.asarray(ln_gamma, np.float32)
    ln_beta = np.asarray(ln_beta, np.float32)
    v = np.asarray(v, np.float32)
    g = np.asarray(g, np.float32)
    bias = np.asarray(bias, np.float32)
    starts = np.asarray(band_start).astype(np.int64)
    widths = np.asarray(band_width).astype(np.int64)

    bf = ml_dtypes.bfloat16
    Wg, bias2, maskn = _fold_weights(ln_gamma, ln_beta, v, g, bias, starts, widths)
    chunk_groups, wcols = _plan_chunks(starts, widths)
    bgroups = _plan_bgroups(starts, widths, chunk_groups)
    Wt = _pack_wt(Wg, chunk_groups, wcols).astype(bf)
    mk = np.ascontiguousarray(
        maskn.reshape(NC, 128, NB).transpose(1, 0, 2).reshape(128, NC * NB)
    ).astype(bf)
    b2 = np.broadcast_to(bias2.reshape(1, ZW), (128, ZW)).astype(bf)
    xk = _prep_x(x)

    key = (tuple(starts.tolist()), tuple(widths.tolist()))
    if key not in _PLAN_CACHE:
        _PLAN_CACHE[key] = _build_program(chunk_groups, wcols, bgroups, widths)
    nc = _PLAN_CACHE[key]

    in_maps = [{"xk": xk[i], "wt": Wt, "mk": mk, "b2": b2} for i in range(NCORES)]
    res = run_bass_kernel_spmd(nc, in_maps, core_ids=list(range(NCORES)))
    LAST_EXEC_NS = res.exec_time_ns

    zarr = np.stack([np.asarray(r["out"]) for r in res.results]).astype(np.float32)
    # [NCORES, NSLAB(b,t2), 128, NB*E] -> [B, NB, T, E]
    z = zarr.reshape(NCORES, B, 2, 128, NB, E)
    z = z.transpose(1, 4, 0, 2, 3, 5).reshape(B, NB, T, E)
    return np.ascontiguousarray(z)


# revision 26
# speedup vs baseline: 1.0093x; 1.0093x over previous
"""Trainium2 Bass kernel for BandSplitModule (masked LN per band + weight-normed Linear).

Strategy (v2 — LN folded into the matmul, K-major everything):
  - Data-parallel over T (2048 = 8 cores x 256). No collectives.
  - Host folds weight-norm + LN affine into W2 = (g*v/||v||)*(gamma*mask),
    bias2 = W@(beta*mask) + bias, then CENTERS the weights:
    W' = W2 - outer(rowsum(W2)/n, mask) so that W'@x = W2@(x - mean).
    Device computes z = rs * (W' @ x) + bias2 with rs = 1/sqrt(var+eps).
  - x is shipped K-major: [33 chunks, 128 k, B*TLOC] bf16. The main matmul
    uses x chunks as the PE stationary (out [t, E] per band), so the final
    scale by rs[t] is a per-partition scalar op.
  - Per-band stats are batched on the TensorEngine: mask/n-stationary
    matmuls stream x and x*x, accumulating Sx=[37,1024], Sxx=[37,1024]
    in PSUM across all 33 chunks. var = Sxx - Sx^2, rs = 1/sqrt(var+eps),
    then 8 tiny PE transposes give rs in [t,37] layout.
  - Output drains (psum->sbuf, *rs) round-robin across Scalar/Vector/GpSimd;
    bias2 is added with one whole-slab [128, 37*128] vector add per slab.
  - Runtime band_start/band_width are baked into the compiled program
    (compilation happens inside kernel(); results cached per band structure).
"""
import numpy as np

B, C, F, T, E = 4, 2, 1025, 2048, 128
MAX_BW = 65
NB = 37
EPS = 1e-5
NCORES = 8
TLOC = T // NCORES  # 256
D = C * MAX_BW * 2  # 260
KF = 4 * F  # 4100 global features (freq-major: k = 4f + 2c + r)
NC = 33  # k chunks of 128
K = NC * 128  # 4224 padded
NSLAB = B * (TLOC // 128)  # 8 output slabs per core
ZW = NB * E  # 4736 output slab width

LAST_EXEC_NS = None

_PLAN_CACHE = {}


def _ensure_trace_hook():
    """Install the antenv.axon_hooks NTFF-profile shim (missing on this image)
    so run_bass_kernel_spmd(trace=True) can capture HW exec time. Fully
    optional — any failure leaves the plain execution path untouched."""
    try:
        import sys, types

        if "antenv.axon_hooks" not in sys.modules:
            mod = types.ModuleType("antenv.axon_hooks")
            _h = {"hook": None}
            mod.set_axon_ntff_profile_hook = lambda h: _h.__setitem__("hook", h)
            mod.get_axon_ntff_profile_hook = lambda: _h["hook"]
            sys.modules["antenv.axon_hooks"] = mod
            try:
                import antenv

                antenv.axon_hooks = mod
            except Exception:
                pass
            try:
                from trn_agent_boot.trn_boot import _ntff_profile_via_ctypes

                hook = _ntff_profile_via_ctypes("/opt/axon/libaxon_pjrt.so")
                if hook is not None:
                    mod.set_axon_ntff_profile_hook(hook)
            except Exception:
                pass
        import concourse.bass_utils as bu

        if not getattr(bu, "_offline_upload_patch", False):
            bu.upload_artifacts = lambda tmpdir: tmpdir
            bu._offline_upload_patch = True
    except Exception:
        pass


def _feature_perm():
    # new index (k,c,r) -> reference index (c,k,r), within one band
    kk, cc, rr = np.meshgrid(
        np.arange(MAX_BW), np.arange(C), np.arange(2), indexing="ij"
    )
    new_i = (kk * 4 + cc * 2 + rr).reshape(-1)
    src_i = (cc * (MAX_BW * 2) + kk * 2 + rr).reshape(-1)
    perm = np.empty(D, np.int64)
    perm[new_i] = src_i
    return perm


def _band_rows(starts, widths):
    """Per band: the global k rows its (clipped) features map to.
    Returns list of arrays rows[n] of length 4*w_n (duplicates where the
    reference's freq clip at F-1 folds several kk onto the same row)."""
    rows = []
    for s, w in zip(starts, widths):
        kk = np.arange(int(w))
        f = np.clip(int(s) + kk, 0, F - 1)
        r4 = (4 * f[:, None] + np.arange(4)[None, :]).reshape(-1)  # (k-major)
        rows.append(r4)
    return rows


def _fold_weights(ln_gamma, ln_beta, v, g, bias, starts, widths):
    """Returns Wg [K, NB*E] f32 (centered, k-major global rows), bias2 [NB,E],
    maskn [K, NB] f32 (1/n per valid row)."""
    karr = np.arange(MAX_BW)
    bw_mask = karr[None, :] < widths[:, None]
    fm = (
        np.broadcast_to(bw_mask[:, None, :, None], (NB, C, MAX_BW, 2))
        .reshape(NB, D)
        .astype(np.float32)
    )
    vnorm = np.sqrt((v * v).sum(-1, keepdims=True))
    vnorm = np.where(vnorm == 0, 1.0, vnorm)
    W = g[..., None] * v / vnorm
    W2 = W * (ln_gamma * fm)[:, None, :]
    bias2 = np.einsum("ned,nd->ne", W, ln_beta * fm) + bias
    W2p = W2[:, :, _feature_perm()]  # [NB, E, D] in (k,c,r) order

    rows = _band_rows(starts, widths)
    Wg = np.zeros((K, NB * E), np.float32)
    maskn = np.zeros((K, NB), np.float32)
    for n in range(NB):
        w = int(widths[n])
        if w == 0:
            continue
        nfeat = float(4 * w)
        wsum2 = W2p[n, :, : 4 * w].sum(axis=1)  # [E]
        Wc = W2p[n, :, : 4 * w].T - wsum2[None, :] / nfeat  # [4w, E] centered
        np.add.at(Wg, (rows[n], slice(n * E, (n + 1) * E)), Wc)
        np.add.at(maskn, (rows[n], n), 1.0 / nfeat)
    return Wg, bias2, maskn


def _plan_chunks(starts, widths):
    """Per 128-row chunk: matmul groups [(wcol, ncols, bands, start, stop)].
    Bands fully inside a chunk are merged pairwise (psum col range is 256)."""
    ranges = []
    for s, w in zip(starts, widths):
        lo = 4 * min(int(s), F - 1)
        hi = 4 * min(int(s) + int(w), F)
        ranges.append((lo, hi))
    chunk_groups = []
    wcol = 0
    for c in range(NC):
        clo, chi = 128 * c, 128 * c + 128
        groups = []
        run = []  # accumulating mergeable full bands
        for n in range(NB):
            lo, hi = ranges[n]
            if hi <= lo or hi <= clo or lo >= chi:
                continue
            full = lo >= clo and hi <= chi
            # merge only full bands that share the same 4-band psum tile
            if full and len(run) < 2 and (not run or run[-1] // 4 == n // 4):
                run.append(n)
                continue
            if run:
                groups.append((wcol, 128 * len(run), tuple(run), True, True))
                wcol += 128 * len(run)
                run = []
            if full:
                run.append(n)
            else:
                groups.append((wcol, 128, (n,), lo >= clo, hi <= chi))
                wcol += 128
        if run:
            groups.append((wcol, 128 * len(run), tuple(run), True, True))
            wcol += 128 * len(run)
        chunk_groups.append(groups)
    return chunk_groups, wcol


def _plan_bgroups(starts, widths, chunk_groups):
    """Split bands into <=32-band groups with contiguous chunk ranges so
    stats/rs/z pipelines per group. Returns list of dicts."""
    ranges = []
    for s, w in zip(starts, widths):
        lo = 4 * min(int(s), F - 1)
        hi = 4 * min(int(s) + int(w), F)
        ranges.append((max(0, lo // 128), max(0, (hi + 127) // 128 - 1)))
    # three band groups (sizes <= 32 so they fit PE quadrant col offsets)
    cuts = [0, 20, 32, NB]
    bgroups = [list(range(cuts[i], cuts[i + 1])) for i in range(3) if cuts[i] < cuts[i + 1]]
    out = []
    for gi, bands in enumerate(bgroups):
        c0 = min(ranges[n][0] for n in bands)
        c1 = max(ranges[n][1] for n in bands)
        out.append(
            dict(
                gi=gi,
                bands=bands,
                nb=len(bands),
                b0=bands[0],
                poff=32 * gi,  # partition offset in shared stats tiles
                c0=c0,
                c1=c1,
            )
        )
    assert len(out) <= 4
    return out


def _pack_wt(Wg, chunk_groups, wcols):
    Wt = np.zeros((128, wcols), np.float32)
    for c, groups in enumerate(chunk_groups):
        sl = Wg[128 * c : 128 * c + 128]
        for wcol, ncols, bands, _, _ in groups:
            off = wcol
            for n in bands:
                Wt[:, off : off + E] = sl[:, n * E : (n + 1) * E]
                off += E
    return Wt


def _prep_x(x):
    """x [B,C,F,T,2] f32 -> [NCORES, NC, 128, B*TLOC] bf16 (k-major chunks)."""
    import ml_dtypes

    xr = np.ascontiguousarray(x.transpose(2, 1, 4, 0, 3)).reshape(KF, B, T)
    xk = np.zeros((K, B, T), ml_dtypes.bfloat16)
    xk[:KF] = xr
    # cols per core: b-major (b*TLOC + t)
    xk = xk.reshape(K, B, NCORES, TLOC).transpose(2, 0, 1, 3)
    xk = np.ascontiguousarray(xk.reshape(NCORES, NC, 128, B * TLOC))
    return xk


class _Balance:
    """Greedy codegen-time engine load balancer (cost in ns, approximate)."""

    def __init__(self):
        self.load = {"v": 0.0, "s": 0.0, "g": 0.0}

    def pick(self, costs):
        eng = min(costs, key=lambda e: self.load[e] + costs[e])
        self.load[eng] += costs[eng]
        return eng


def _build_program(chunk_groups, wcols, bgroups, widths):
    import concourse.bass as bass
    import concourse.bacc as bacc
    import concourse.tile as tile
    from concourse import mybir
    from concourse.masks import make_identity
    from contextlib import ExitStack

    f32 = mybir.dt.float32
    bf16 = mybir.dt.bfloat16
    TC = 1024  # B * TLOC columns per core
    nc = bacc.Bacc()
    x_ext = nc.declare_dram_parameter("xk", [NC, 128, TC], bf16, isOutput=False)
    wt_ext = nc.declare_dram_parameter("wt", [128, wcols], bf16, isOutput=False)
    mk_ext = nc.declare_dram_parameter("mk", [128, NC * NB], bf16, isOutput=False)
    b2_ext = nc.declare_dram_parameter("b2", [128, ZW], bf16, isOutput=False)
    z_ext = nc.declare_dram_parameter("out", [NSLAB, 128, ZW], bf16, isOutput=True)

    bal = _Balance()

    with ExitStack() as ctx:
        tc = ctx.enter_context(tile.TileContext(nc))
        consts = ctx.enter_context(tc.tile_pool(name="consts", bufs=1))
        xpool = ctx.enter_context(tc.tile_pool(name="x", bufs=1))
        x2pool = ctx.enter_context(tc.tile_pool(name="x2", bufs=6))
        rspool = ctx.enter_context(tc.tile_pool(name="rs", bufs=1))
        zspool = ctx.enter_context(tc.tile_pool(name="zs", bufs=4))
        st_psum = ctx.enter_context(tc.tile_pool(name="st", bufs=1, space="PSUM"))
        z_psum = ctx.enter_context(tc.tile_pool(name="zq", bufs=5, space="PSUM"))
        rs_psum = ctx.enter_context(tc.tile_pool(name="rp", bufs=1, space="PSUM"))

        ident = consts.tile([128, 128], f32)
        make_identity(nc, ident)
        eps_t = consts.tile([128, 1], f32)
        nc.vector.memset(eps_t, EPS)

        # DMA order: mask first (stats lhsT), then the first group's x chunks,
        # then weights/bias, then the remaining x chunks.
        mk_sb = consts.tile([128, NC * NB], bf16)
        nc.sync.dma_start(out=mk_sb, in_=mk_ext[:, :])
        xt = [None] * NC
        early = list(range(bgroups[0]["c0"], bgroups[0]["c1"] + 1))
        order = early + [c for c in range(NC) if c not in early]
        for i, c in enumerate(order):
            t = xpool.tile([128, TC], bf16, tag=f"x{c}", name=f"x{c}")
            nc.sync.dma_start(out=t, in_=x_ext[c])
            xt[c] = t
            if i == 3:
                wt_sb = consts.tile([128, wcols], bf16)
                nc.sync.dma_start(out=wt_sb, in_=wt_ext[:, :])
                b2_sb = consts.tile([128, ZW], bf16)
                nc.sync.dma_start(out=b2_sb, in_=b2_ext[:, :])

        # Shared stats psum tiles (one column-half at a time); group gi uses
        # partitions [32*gi, 32*gi+nb)
        Sx = st_psum.tile([128, 512], f32, tag="Sx")
        Sxx = st_psum.tile([128, 512], f32, tag="Sxx")
        var_t = rspool.tile([128, TC], f32, tag="var")
        sqr_t = rspool.tile([128, TC], f32, tag="sqr")

        def emit_stats(g, h):
            poff, nb, b0 = g["poff"], g["nb"], g["b0"]
            hs = slice(512 * h, 512 * h + 512)
            for c in range(g["c0"], g["c1"] + 1):
                x2 = x2pool.tile([128, 512], bf16, tag="x2", name="x2")
                if c >= NC - 8:
                    eng = "g"  # late chunks: gpsimd has slack by then
                    bal.load["g"] += 1100
                else:
                    eng = bal.pick({"v": 390, "s": 560})
                if eng == "v":
                    nc.vector.tensor_mul(x2, xt[c][:, hs], xt[c][:, hs])
                elif eng == "g":
                    nc.gpsimd.tensor_mul(x2, xt[c][:, hs], xt[c][:, hs])
                else:
                    nc.scalar.square(out=x2, in_=xt[c][:, hs])
                mk_c = mk_sb[:, c * NB + b0 : c * NB + b0 + nb]
                st = c == g["c0"]
                sp = c == g["c1"]
                o = slice(poff, poff + nb)
                nc.tensor.matmul(Sx[o, :], lhsT=mk_c, rhs=xt[c][:, hs], start=st, stop=sp)
                nc.tensor.matmul(Sxx[o, :], lhsT=mk_c, rhs=x2, start=st, stop=sp)

        def emit_rs(g, h):
            """var = Sxx - Sx^2; invrs = sqrt(var+eps); rsT[s] = 1/invrs^T."""
            poff, nb = g["poff"], g["nb"]
            o = slice(poff, poff + nb)
            rsT = []
            sl = slice(512 * h, 512 * h + 512)
            nc.scalar.square(out=var_t[o, sl], in_=Sx[o, :])
            nc.vector.tensor_sub(var_t[o, sl], Sxx[o, :], var_t[o, sl])
            nc.scalar.activation(
                out=sqr_t[o, sl],
                in_=var_t[o, sl],
                func=mybir.ActivationFunctionType.Sqrt,
                bias=eps_t[o, :],
                scale=1.0,
            )
            for s in range(4 * h, 4 * h + 4):
                rp = rs_psum.tile([128, 32], f32, tag="rp", name="rp")
                nc.tensor.transpose(
                    out=rp[:, :nb],
                    in_=sqr_t[o, 128 * s : 128 * s + 128],
                    identity=ident[o, poff : poff + nb],
                )
                rt = rspool.tile([128, 32], f32, tag=f"rsT{g['gi']}_{s}", name="rt")
                nc.vector.reciprocal(out=rt[:, :nb], in_=rp[:, :nb])
                rsT.append(rt)
            return rsT

        def emit_z(g, rsT):
            b0, nb = g["b0"], g["nb"]
            gw = nb * E
            gcol0 = b0 * E
            bandset = set(g["bands"])
            # quads of 4 bands sharing one [128,512] psum tile
            quad_bands = {}
            for n in g["bands"]:
                if widths[n] > 0:
                    quad_bands.setdefault(n // 4, []).append(n)
            # per chunk, entries of this group
            centries = []
            for c in range(g["c0"], g["c1"] + 1):
                ent = [e for e in chunk_groups[c] if e[2][0] in bandset]
                centries.append((c, ent))
            for s in range(NSLAB):
                toff = 128 * s
                zg = zspool.tile([128, gw], bf16, tag=f"zg{g['gi']}", name="zg")
                qtile = {}
                qstops = {q: 0 for q in quad_bands}
                nstops = {q: len(v) for q, v in quad_bands.items()}
                for c, ent in centries:
                    x_c = xt[c][:, toff : toff + 128]
                    for wcol, ncols, bands, st, sp in ent:
                        q = bands[0] // 4
                        if q not in qtile:
                            qtile[q] = z_psum.tile([128, 512], f32, tag="zq", name="zq")
                        zp = qtile[q]
                        pc = (bands[0] % 4) * 128
                        nc.tensor.matmul(
                            zp[:, pc : pc + ncols],
                            lhsT=x_c,
                            rhs=wt_sb[:, wcol : wcol + ncols],
                            start=st,
                            stop=sp,
                        )
                        if sp:
                            qstops[q] += len(bands)
                            if qstops[q] == nstops[q]:
                                # scaled per-band drains psum->sbuf
                                for n in quad_bands[q]:
                                    dst = zg[:, (n - b0) * E : (n - b0 + 1) * E]
                                    src = zp[:, (n % 4) * E : (n % 4) * E + E]
                                    sc = rsT[s][:, n - b0 : n - b0 + 1]
                                    eng = bal.pick({"v": 300, "s": 415})
                                    if eng == "v":
                                        nc.vector.tensor_scalar(
                                            dst, src, sc, None, mybir.AluOpType.mult
                                        )
                                    else:
                                        nc.scalar.mul(out=dst, in_=src, mul=sc)
                # bias + store (gpsimd only for early big slabs, off the tail)
                if g["gi"] == 0 and s < 5:
                    bal.load["g"] += 4300
                    nc.gpsimd.tensor_add(zg, zg, b2_sb[:, gcol0 : gcol0 + gw])
                else:
                    bal.load["v"] += 60 + gw * 0.55
                    nc.vector.tensor_add(zg, zg, b2_sb[:, gcol0 : gcol0 + gw])
                nc.sync.dma_start(out=z_ext[s][:, gcol0 : gcol0 + gw], in_=zg)

        # ---- pipelined emission, stats by column-halves through 2 psum banks:
        # statsA(g), rsA(g), statsB(g), rsB(g), z(g, all slabs), next group...
        for gi, g in enumerate(bgroups):
            rsT = []
            for h in (0, 1):
                emit_stats(g, h)
                rsT += emit_rs(g, h)
            emit_z(g, rsT)
    nc.compile()
    return nc


def kernel(x, ln_gamma, ln_beta, v, g, bias, band_start, band_width):
    global LAST_EXEC_NS
    _ensure_trace_hook()
    from concourse.bass_utils import run_bass_kernel_spmd
    import ml_dtypes

    x = np.asarray(x, np.float32)
    ln_gamma = np.asarray(ln_gamma, np.float32)
    ln_beta = np.asarray(ln_beta, np.float32)
    v = np.asarray(v, np.float32)
    g = np.asarray(g, np.float32)
    bias = np.asarray(bias, np.float32)
    starts = np.asarray(band_start).astype(np.int64)
    widths = np.asarray(band_width).astype(np.int64)

    bf = ml_dtypes.bfloat16
    Wg, bias2, maskn = _fold_weights(ln_gamma, ln_beta, v, g, bias, starts, widths)
    chunk_groups, wcols = _plan_chunks(starts, widths)
    bgroups = _plan_bgroups(starts, widths, chunk_groups)
    Wt = _pack_wt(Wg, chunk_groups, wcols).astype(bf)
    mk = np.ascontiguousarray(
        maskn.reshape(NC, 128, NB).transpose(1, 0, 2).reshape(128, NC * NB)
    ).astype(bf)
    b2 = np.broadcast_to(bias2.reshape(1, ZW), (128, ZW)).astype(bf)
    xk = _prep_x(x)

    key = (tuple(starts.tolist()), tuple(widths.tolist()))
    if key not in _PLAN_CACHE:
        _PLAN_CACHE[key] = _build_program(chunk_groups, wcols, bgroups, widths)
    nc = _PLAN_CACHE[key]

    in_maps = [{"xk": xk[i], "wt": Wt, "mk": mk, "b2": b2} for i in range(NCORES)]
    res = run_bass_kernel_spmd(nc, in_maps, core_ids=list(range(NCORES)))
    LAST_EXEC_NS = res.exec_time_ns

    zarr = np.stack([np.asarray(r["out"]) for r in res.results]).astype(np.float32)
    # [NCORES, NSLAB(b,t2), 128, NB*E] -> [B, NB, T, E]
    z = zarr.reshape(NCORES, B, 2, 128, NB, E)
    z = z.transpose(1, 4, 0, 2, 3, 5).reshape(B, NB, T, E)
    return np.ascontiguousarray(z)
